# revision 3
# baseline (speedup 1.0000x reference)
"""Causal linear attention (chunked scan) for Trainium2, 8 NeuronCores.

Problem: B=4, T=2048, E=1024, H=16 heads, D=64, CHUNK=128.
  qkv = x @ w_attn.T ; q,k -> phi = elu+1 ; chunked causal linear attention
  with running state S[D,D], z[D] per (b,h); out = y @ w_proj.T.

Sharding: core = b*2 + hg  (b in 0..3 batches, hg in 0..1 half-of-heads).
Each core computes its batch's tokens against its 8 heads; the host sums the
two half-head partial outputs per batch. No cross-core traffic.

qkv projection runs as an error-compensated fp8 DoubleRow product (2x PE
throughput, 3 of 4 hi/lo cross terms): with X=x, W=64*w and fp8 casts
  X8=f8(X), Xr4=f8(4(X-X8)), X8q=f8(X/4); W8=f8(W), Wr4=f8(4(W-W8)),
  W8q=f8(W/4):
  X.W ~= X8.W8 + Xr4.W8q + X8q.Wr4   (all three terms at PSUM scale 64*x.w,
so they accumulate in one PSUM group; the dropped Xr.Wr term is ~0.1%).
Attention interior and the output projection run in f16 (halves the bf16
rounding noise; same PE cost). Output DMA'd as f16, summed f32 on host.

Single interleaved pipeline per core (emission order = PE order):
  proj(tb0), then for tb: proj(tb) followed by scan chunks of tb-1, with the
  output-projection block of chunk n-1 emitted inside chunk n as PE filler
  while the DVE mask/cast of chunk n runs.

DMA: SP issues startup-critical loads + output stores (HWDGE ~630ns serial
each — count kept low); the Pool engine issues the bulk residual/quarter
tensors via SWDGE so HWDGE doesn't serialize against them.
"""

import sys

if "/opt/trn_rl_repo" not in sys.path:
    sys.path.insert(0, "/opt/trn_rl_repo")

import numpy as np
import ml_dtypes

B, T, E = 4, 2048, 1024
H, D, CH = 16, 64, 128
NCH = T // CH            # 16 chunks
HPC = H // 2             # 8 heads per core
EB = E // 128            # 8 contraction blocks
FB_QK = 8                # feature blocks for q|k (1024 features)
TB = 4                   # token blocks of 512 for the A-part
N_CORES = 8
WS = 64.0                # weight prescale for fp8 range

_cache = {}


def _build():
    import concourse.bacc as bacc
    import concourse.tile as tile
    from concourse import mybir

    f32 = mybir.dt.float32
    f16 = mybir.dt.float16
    f8 = mybir.dt.float8e4
    AF = mybir.ActivationFunctionType
    ALU = mybir.AluOpType
    DR = mybir.MatmulPerfMode.DoubleRow

    nc = bacc.Bacc("TRN2", target_bir_lowering=False, debug=False, num_devices=N_CORES)

    x8 = nc.dram_tensor("x8", [E, T], f8, kind="ExternalInput")
    xr4 = nc.dram_tensor("xr4", [E, T], f8, kind="ExternalInput")
    x8q = nc.dram_tensor("x8q", [E, T], f8, kind="ExternalInput")
    wA8 = nc.dram_tensor("wA8", [E, 1024], f8, kind="ExternalInput")
    wAr4 = nc.dram_tensor("wAr4", [E, 1024], f8, kind="ExternalInput")
    wA8q = nc.dram_tensor("wA8q", [E, 1024], f8, kind="ExternalInput")
    wB8 = nc.dram_tensor("wB8", [E, 512], f8, kind="ExternalInput")
    wBr4 = nc.dram_tensor("wBr4", [E, 512], f8, kind="ExternalInput")
    wB8q = nc.dram_tensor("wB8q", [E, 512], f8, kind="ExternalInput")
    wpT = nc.dram_tensor("wpT", [512, E], f16, kind="ExternalInput")
    maskT = nc.dram_tensor("maskT", [CH, HPC * CH], f16, kind="ExternalInput")
    ident = nc.dram_tensor("ident", [CH, CH], f16, kind="ExternalInput")
    out = nc.dram_tensor("out", [T, E], f16, kind="ExternalOutput")

    with tile.TileContext(nc) as tc:
        with tc.tile_pool(name="main", bufs=1) as main, \
             tc.tile_pool(name="phist", bufs=4) as phist, \
             tc.tile_pool(name="scmp", bufs=4) as scmp, \
             tc.tile_pool(name="st2", bufs=4) as st2, \
             tc.tile_pool(name="ost", bufs=4) as ost, \
             tc.tile_pool(name="psProj", bufs=2, space="PSUM") as psProj, \
             tc.tile_pool(name="psTp", bufs=1, space="PSUM") as psTp, \
             tc.tile_pool(name="psSd", bufs=1, space="PSUM") as psSd, \
             tc.tile_pool(name="psS", bufs=2, space="PSUM") as psS, \
             tc.tile_pool(name="psN", bufs=2, space="PSUM") as psN:
            qkT = main.tile([128, FB_QK * T], f16)
            k_tok = main.tile([128, NCH * 512], f16)
            v_aug = main.tile([128, NCH * HPC * 65], f16)
            mask_sb = main.tile([128, HPC * CH], f16)
            id_sb = main.tile([128, CH], f16)
            S_sb = main.tile([128, 4 * 128], f16)
            S_f32 = main.tile([128, 4 * 128], f32)
            yT_all = main.tile([128, 4 * T], f16)
            wp_sb = main.tile([128, 4 * 1024], f16)
            x8_sb = main.tile([128, EB * T], f8)
            xr4_sb = main.tile([128, EB * T], f8)
            x8q_sb = main.tile([128, EB * T], f8)
            wA8_sb = main.tile([128, EB * 1024], f8)
            wAr4_sb = main.tile([128, EB * 1024], f8)
            wA8q_sb = main.tile([128, EB * 1024], f8)
            wB8_sb = main.tile([128, EB * 512], f8)
            wBr4_sb = main.tile([128, EB * 512], f8)
            wB8q_sb = main.tile([128, EB * 512], f8)

            nc.sync.dma_start(mask_sb[:], maskT[:])
            nc.sync.dma_start(id_sb[:], ident[:])
            nc.vector.memset(S_sb[:], 0.0)
            nc.vector.memset(S_f32[:], 0.0)
            ones_view = v_aug.rearrange("p (n h e) -> p n h e", h=HPC, e=65)[:, :, :, 64]
            nc.vector.memset(ones_view, 1.0)

            # DMA order: term-1 operands first (wB8, x8 tb0, wA8), then
            # term-2/3 so PE's per-unit stalls shorten progressively. Bulk
            # residual/quarter x and wA tensors go via Pool SWDGE.
            for eb in range(EB):
                nc.sync.dma_start(wB8_sb[:, eb * 512:(eb + 1) * 512],
                                  wB8[eb * 128:(eb + 1) * 128, :])
                nc.sync.dma_start(x8_sb[:, eb * T: eb * T + 512],
                                  x8[eb * 128:(eb + 1) * 128, 0:512])
            for eb in range(EB):
                nc.sync.dma_start(wB8q_sb[:, eb * 512:(eb + 1) * 512],
                                  wB8q[eb * 128:(eb + 1) * 128, :])
            for eb in range(EB):
                nc.sync.dma_start(xr4_sb[:, eb * T: eb * T + T],
                                    xr4[eb * 128:(eb + 1) * 128, :])
            for eb in range(EB):
                nc.sync.dma_start(wBr4_sb[:, eb * 512:(eb + 1) * 512],
                                  wBr4[eb * 128:(eb + 1) * 128, :])
            for eb in range(EB):
                nc.sync.dma_start(x8q_sb[:, eb * T: eb * T + T],
                                    x8q[eb * 128:(eb + 1) * 128, :])
            for eb in range(EB):
                nc.sync.dma_start(wA8_sb[:, eb * 1024:(eb + 1) * 1024],
                                  wA8[eb * 128:(eb + 1) * 128, :])
            for eb in range(EB):
                nc.sync.dma_start(wA8q_sb[:, eb * 1024:(eb + 1) * 1024],
                                    wA8q[eb * 128:(eb + 1) * 128, :])
            for eb in range(EB):
                nc.sync.dma_start(wAr4_sb[:, eb * 1024:(eb + 1) * 1024],
                                    wAr4[eb * 128:(eb + 1) * 128, :])
            for tb in range(1, TB):
                for eb in range(EB):
                    nc.sync.dma_start(
                        x8_sb[:, eb * T + tb * 512: eb * T + (tb + 1) * 512],
                        x8[eb * 128:(eb + 1) * 128, tb * 512:(tb + 1) * 512])
            nc.sync.dma_start(wp_sb.rearrange("p (hp t) -> p hp t", hp=4),
                              wpT.rearrange("(hp p) t -> p hp t", p=128))

            # [p, eb, cols] views for DoubleRow k-tile pairs (dim1 = 2 slices)
            x8v = x8_sb.rearrange("p (eb t) -> p eb t", eb=EB)
            xr4v = xr4_sb.rearrange("p (eb t) -> p eb t", eb=EB)
            x8qv = x8q_sb.rearrange("p (eb t) -> p eb t", eb=EB)
            wA8v = wA8_sb.rearrange("p (eb c) -> p eb c", eb=EB)
            wAr4v = wAr4_sb.rearrange("p (eb c) -> p eb c", eb=EB)
            wA8qv = wA8q_sb.rearrange("p (eb c) -> p eb c", eb=EB)
            wB8v = wB8_sb.rearrange("p (eb c) -> p eb c", eb=EB)
            wBr4v = wBr4_sb.rearrange("p (eb c) -> p eb c", eb=EB)
            wB8qv = wB8q_sb.rearrange("p (eb c) -> p eb c", eb=EB)

            A_TERMS = [(wA8v, x8v), (wA8qv, xr4v), (wAr4v, x8qv)]
            B_TERMS = [(x8v, wB8v), (xr4v, wB8qv), (x8qv, wBr4v)]

            def proj_units(tb):
                units = []

                def a_unit(fb, tb=tb):
                    ps = psProj.tile([128, 512], f32, name="psa", tag="proj")
                    idx = 0
                    for wv, xv in A_TERMS:
                        for i in range(4):
                            nc.tensor.matmul(
                                ps[:],
                                wv[:, 2 * i:2 * i + 2, fb * 128:(fb + 1) * 128],
                                xv[:, 2 * i:2 * i + 2, tb * 512:(tb + 1) * 512],
                                start=(idx == 0), stop=(idx == 11), perf_mode=DR)
                            idx += 1
                    # phi(x) = relu(x) + min(exp(x), 1); PSUM holds 64*x
                    ex = phist.tile([128, 512], f16, name="ex", tag="ex")
                    nc.scalar.activation(ex[:], ps[:], AF.Exp, scale=1.0 / WS)
                    rl = phist.tile([128, 512], f16, name="rl", tag="rl")
                    nc.scalar.activation(rl[:], ps[:], AF.Relu, scale=1.0 / WS)
                    dst = qkT[:, fb * T + tb * 512: fb * T + tb * 512 + 512]
                    nc.vector.scalar_tensor_tensor(
                        dst, ex[:], 1.0, rl[:], ALU.min, ALU.add)

                def b_unit(n):
                    ps = psProj.tile([128, 512], f32, name="psb", tag="proj")
                    idx = 0
                    for xv, wv in B_TERMS:
                        for i in range(4):
                            nc.tensor.matmul(
                                ps[:],
                                xv[:, 2 * i:2 * i + 2, n * CH:(n + 1) * CH],
                                wv[:, 2 * i:2 * i + 2, :],
                                start=(idx == 0), stop=(idx == 11), perf_mode=DR)
                            idx += 1
                    src_ = ps.rearrange("p (h e) -> p h e", e=64)
                    dst = v_aug[:, n * HPC * 65:(n + 1) * HPC * 65] \
                        .rearrange("p (h e) -> p h e", e=65)[:, :, 0:64]
                    nc.scalar.mul(dst, src_, 1.0 / WS)

                def kt_unit(n):
                    kt = psTp.tile([128, 512], f16, name="ktp", tag="tp")
                    for fb4 in range(4):
                        nc.tensor.transpose(
                            kt[:, fb4 * 128:(fb4 + 1) * 128],
                            qkT[:, (4 + fb4) * T + n * CH:(4 + fb4) * T + n * CH + CH],
                            id_sb[:])
                    nc.vector.tensor_copy(
                        k_tok[:, n * 512:(n + 1) * 512], kt[:])

                if tb == 0:
                    for n in range(4):
                        units.append(lambda n=n: b_unit(n))
                    for fb in range(FB_QK):
                        units.append(lambda fb=fb: a_unit(fb))
                else:
                    for fb in range(FB_QK):
                        units.append(lambda fb=fb: a_unit(fb))
                    for n in range(tb * 4, tb * 4 + 4):
                        units.append(lambda n=n: b_unit(n))
                for n in range(tb * 4, tb * 4 + 4):
                    units.append(lambda n=n: kt_unit(n))
                return units

            def emit_p3(n):
                # output projection for token block n (needs yT_all of chunk n)
                for eo in range(2):
                    po = psProj.tile([128, 512], f32, name="pop", tag="proj")
                    for hp in range(4):
                        nc.tensor.matmul(
                            po[:],
                            yT_all[:, hp * T + n * CH: hp * T + n * CH + CH],
                            wp_sb[:, hp * 1024 + eo * 512: hp * 1024 + eo * 512 + 512],
                            start=(hp == 0), stop=(hp == 3))
                    ob = ost.tile([128, 512], f16, name="ob", tag="ob")
                    nc.scalar.copy(ob[:], po[:])
                    nc.sync.dma_start(
                        out[n * CH:(n + 1) * CH, eo * 512:(eo + 1) * 512], ob[:])

            def emit_chunk(n, filler=None):
                # scoresT, one [128,512] tile per parity half (bank-pure
                # row-groups; 2-buf rotation pipelines across chunks)
                scm = scmp.tile([128, HPC * CH], f16, name="scm", tag="scm")
                scgs = [psS.tile([128, 512], f32, name=f"scg{g}", tag="scg")
                        for g in range(2)]
                for hp in range(4):
                    for g in range(2):  # alternate row groups -> PE overlaps
                        b64 = g * 64
                        nc.tensor.matmul(
                            scgs[g][:, hp * CH:(hp + 1) * CH],
                            qkT[b64:b64 + 64,
                                (4 + hp) * T + n * CH:(4 + hp) * T + n * CH + CH],
                            qkT[b64:b64 + 64, hp * T + n * CH: hp * T + n * CH + CH],
                            start=True, stop=True)
                for g in range(2):
                    nc.vector.tensor_mul(scm[:, g * 512:(g + 1) * 512], scgs[g][:],
                                         mask_sb[:, g * 512:(g + 1) * 512])

                # PE filler while the mask/cast runs on DVE
                if n >= 1:
                    emit_p3(n - 1)
                if filler is not None:
                    filler()

                # num = scores @ v_aug + q @ S_aug (per parity half)
                nmb_g = []
                for g in range(2):
                    nmg = psN.tile([128, 512], f32, name="nmg", tag="nmg")
                    nmb_g.append(nmg)
                    for hp in range(4):
                        h = hp * 2 + g
                        b64 = g * 64
                        hc = hp * 128
                        qT_ap = qkT[b64:b64 + 64, hp * T + n * CH: hp * T + n * CH + CH]
                        va_ap = v_aug[:, (n * HPC + h) * 65:(n * HPC + h) * 65 + 65]
                        nc.tensor.matmul(nmg[:, hc: hc + 65],
                                         scm[:, g * 512 + hc: g * 512 + hc + CH], va_ap,
                                         start=True, stop=False)
                        nc.tensor.matmul(nmg[:, hc: hc + 65], qT_ap,
                                         S_sb[b64:b64 + 64, hp * 128: hp * 128 + 65],
                                         start=False, stop=True)

                # state deltas + f32 accumulate + f16 snapshot (snap on Pool)
                sdb = psSd.tile([128, 512], f32, name="sdb", tag="sdb")
                for h in range(HPC):
                    b64 = (h % 2) * 64
                    hp = h // 2
                    va_ap = v_aug[:, (n * HPC + h) * 65:(n * HPC + h) * 65 + 65]
                    nc.tensor.matmul(
                        sdb[b64:b64 + 64, hp * 128: hp * 128 + 65],
                        k_tok[:, n * 512 + h * 64: n * 512 + h * 64 + 64],
                        va_ap, start=True, stop=True)
                Sf_v = S_f32.rearrange("p (g e) -> p g e", e=CH)[:, :, 0:65]
                Sb_v = S_sb.rearrange("p (g e) -> p g e", e=CH)[:, :, 0:65]
                sd_v = sdb.rearrange("p (g e) -> p g e", e=CH)[:, :, 0:65]
                nc.vector.tensor_add(Sf_v, Sf_v, sd_v)
                nc.vector.tensor_copy(Sb_v, Sf_v)

                # y = num / den, per parity half
                yb = st2.tile([128, 512], f16, name="yb", tag="yb")
                yb_v = yb.rearrange("p (hh two e) -> p hh two e", two=2, e=64)
                for g in range(2):
                    half = nmb_g[g].rearrange("p (hh e) -> p hh e", e=CH)
                    rcp = st2.tile([128, 4], f32, name=f"rcp{g}", tag=f"rcp{g}")
                    nc.vector.reciprocal(rcp[:], half[:, :, 64])
                    nc.vector.tensor_mul(
                        yb_v[:, :, g, :],
                        half[:, :, 0:64],
                        rcp[:, :, None].broadcast_to([128, 4, 64]))

                # yT via PE transpose (head pairs) -> yT_all
                ytp = psTp.tile([128, 512], f16, name="ytp", tag="tp")
                for hp in range(4):
                    nc.tensor.transpose(ytp[:, hp * 128:(hp + 1) * 128],
                                        yb[:, hp * CH:(hp + 1) * CH], id_sb[:])
                nc.vector.tensor_copy(
                    yT_all.rearrange("p (hp t) -> p hp t", hp=4)[:, :, n * CH:(n + 1) * CH],
                    ytp.rearrange("p (hp e) -> p hp e", hp=4))

            for u in proj_units(0):
                u()
            units = []
            for tb in range(1, TB):
                units.extend(proj_units(tb))
            state = {"ui": 0}

            def pace(target):
                while state["ui"] < min(target, len(units)):
                    units[state["ui"]]()
                    state["ui"] += 1

            for n in range(NCH):
                pace(16 * (n // 4))          # hard dep: chunk n needs its tb
                emit_chunk(n, lambda: pace(7 * (n + 1)))
            emit_p3(NCH - 1)

    nc.compile()
    return nc


def _split3(a, hi_dt=ml_dtypes.float8_e4m3fn):
    """fp8 hi / 4x-residual / quarter triplet of a float32 array."""
    a8 = a.astype(hi_dt)
    ar4 = ((a - a8.astype(np.float32)) * 4.0).astype(hi_dt)
    a8q = (a * 0.25).astype(hi_dt)
    return a8, ar4, a8q


def _prep_core_inputs(x, w_attn, w_proj, core):
    b, hg = core // 2, core % 2
    s = slice(hg * 512, (hg + 1) * 512)
    xT = np.ascontiguousarray(x[b].T)
    wA = np.ascontiguousarray(
        np.concatenate([w_attn[s, :], w_attn[E + hg * 512: E + (hg + 1) * 512, :]],
                       0).T) * WS
    wB = np.ascontiguousarray(
        w_attn[2 * E + hg * 512: 2 * E + (hg + 1) * 512, :].T) * WS
    x8, xr4, x8q = _split3(xT)
    wA8, wAr4, wA8q = _split3(wA)
    wB8, wBr4, wB8q = _split3(wB)
    wpT_ = np.ascontiguousarray(w_proj[:, s].T).astype(np.float16)
    mask1 = np.triu(np.ones((CH, CH), dtype=np.float32))
    maskT = np.tile(mask1, (1, HPC)).astype(np.float16)
    ident = np.eye(CH, dtype=np.float32).astype(np.float16)
    return {"x8": x8, "xr4": xr4, "x8q": x8q,
            "wA8": wA8, "wAr4": wAr4, "wA8q": wA8q,
            "wB8": wB8, "wBr4": wBr4, "wB8q": wB8q,
            "wpT": wpT_, "maskT": maskT, "ident": ident}


def kernel(x, w_attn, w_proj, _trace=False):
    from concourse.bass_utils import run_bass_kernel_spmd

    if "nc" not in _cache:
        _cache["nc"] = _build()
    nc = _cache["nc"]

    x = np.asarray(x, dtype=np.float32)
    w_attn = np.asarray(w_attn, dtype=np.float32)
    w_proj = np.asarray(w_proj, dtype=np.float32)

    in_maps = [_prep_core_inputs(x, w_attn, w_proj, c) for c in range(N_CORES)]
    res = run_bass_kernel_spmd(nc, in_maps, core_ids=list(range(N_CORES)),
                               trace=_trace)
    _cache["last_results"] = res

    out = np.empty((B, T, E), dtype=np.float32)
    for b in range(B):
        out[b] = (res.results[2 * b]["out"].astype(np.float32)
                  + res.results[2 * b + 1]["out"].astype(np.float32))
    return out


# revision 4
# speedup vs baseline: 1.1566x; 1.1566x over previous
"""Causal linear attention (chunked scan) for Trainium2, 8 NeuronCores.

Problem: B=4, T=2048, E=1024, H=16 heads, D=64, CHUNK=128.
  qkv = x @ w_attn.T ; q,k -> phi = elu+1 ; chunked causal linear attention
  with running state S[D,D], z[D] per (b,h); out = y @ w_proj.T.

Sharding: core = b*2 + hg  (b in 0..3 batches, hg in 0..1 half-of-heads).
Each core computes its batch's tokens against its 8 heads; the host sums the
two half-head partial outputs per batch. No cross-core traffic.

qkv projection runs as an error-compensated fp8 DoubleRow product (2x PE
throughput, 3 of 4 hi/lo cross terms): with X=x, W=64*w and fp8 casts
  X8=f8(X), Xr4=f8(4(X-X8)), X8q=f8(X/4); W8=f8(W), Wr4=f8(4(W-W8)),
  W8q=f8(W/4):
  X.W ~= X8.W8 + Xr4.W8q + X8q.Wr4   (all three terms at PSUM scale 64*x.w,
so they accumulate in one PSUM group; the dropped Xr.Wr term is ~0.1%).
Attention interior and the output projection run in f16 (halves the bf16
rounding noise; same PE cost). Output DMA'd as f16, summed f32 on host.

Single interleaved pipeline per core (emission order = PE order):
  proj(tb0), then for tb: proj(tb) followed by scan chunks of tb-1, with the
  output-projection block of chunk n-1 emitted inside chunk n as PE filler
  while the DVE mask/cast of chunk n runs.

DMA: SP issues startup-critical loads + output stores (HWDGE ~630ns serial
each — count kept low); the Pool engine issues the bulk residual/quarter
tensors via SWDGE so HWDGE doesn't serialize against them.
"""

import sys

if "/opt/trn_rl_repo" not in sys.path:
    sys.path.insert(0, "/opt/trn_rl_repo")

import numpy as np
import ml_dtypes

B, T, E = 4, 2048, 1024
H, D, CH = 16, 64, 128
NCH = T // CH            # 16 chunks
HPC = H // 2             # 8 heads per core
EB = E // 128            # 8 contraction blocks
FB_QK = 8                # feature blocks for q|k (1024 features)
TB = 4                   # token blocks of 512 for the A-part
N_CORES = 8
WS = 64.0                # weight prescale for fp8 range

_cache = {}


def _build():
    import concourse.bacc as bacc
    import concourse.tile as tile
    from concourse import mybir

    f32 = mybir.dt.float32
    f16 = mybir.dt.float16
    f8 = mybir.dt.float8e4
    AF = mybir.ActivationFunctionType
    ALU = mybir.AluOpType
    DR = mybir.MatmulPerfMode.DoubleRow

    nc = bacc.Bacc("TRN2", target_bir_lowering=False, debug=False, num_devices=N_CORES)

    x8 = nc.dram_tensor("x8", [E, T], f8, kind="ExternalInput")
    xr4 = nc.dram_tensor("xr4", [E, T], f8, kind="ExternalInput")
    x8q = nc.dram_tensor("x8q", [E, T], f8, kind="ExternalInput")
    wA8 = nc.dram_tensor("wA8", [E, 1024], f8, kind="ExternalInput")
    wAr4 = nc.dram_tensor("wAr4", [E, 1024], f8, kind="ExternalInput")
    wA8q = nc.dram_tensor("wA8q", [E, 1024], f8, kind="ExternalInput")
    wB8 = nc.dram_tensor("wB8", [E, 512], f8, kind="ExternalInput")
    wBr4 = nc.dram_tensor("wBr4", [E, 512], f8, kind="ExternalInput")
    wB8q = nc.dram_tensor("wB8q", [E, 512], f8, kind="ExternalInput")
    wpT = nc.dram_tensor("wpT", [512, E], f16, kind="ExternalInput")
    maskT = nc.dram_tensor("maskT", [CH, HPC * CH], f16, kind="ExternalInput")
    ident = nc.dram_tensor("ident", [CH, CH], f16, kind="ExternalInput")
    out = nc.dram_tensor("out", [T, E], f16, kind="ExternalOutput")

    with tile.TileContext(nc) as tc:
        with tc.tile_pool(name="main", bufs=1) as main, \
             tc.tile_pool(name="phist", bufs=4) as phist, \
             tc.tile_pool(name="scmp", bufs=4) as scmp, \
             tc.tile_pool(name="st2", bufs=4) as st2, \
             tc.tile_pool(name="ost", bufs=4) as ost, \
             tc.tile_pool(name="psProj", bufs=2, space="PSUM") as psProj, \
             tc.tile_pool(name="psTp", bufs=1, space="PSUM") as psTp, \
             tc.tile_pool(name="psSd", bufs=1, space="PSUM") as psSd, \
             tc.tile_pool(name="psS", bufs=2, space="PSUM") as psS, \
             tc.tile_pool(name="psN", bufs=2, space="PSUM") as psN:
            qkT = main.tile([128, FB_QK * T], f16)
            k_tok = main.tile([128, NCH * 512], f16)
            v_aug = main.tile([128, NCH * HPC * 65], f16)
            mask_sb = main.tile([128, HPC * CH], f16)
            id_sb = main.tile([128, CH], f16)
            S_sb = main.tile([128, 4 * 128], f16)
            S_f32 = main.tile([128, 4 * 128], f32)
            yT_all = main.tile([128, 4 * T], f16)
            wp_sb = main.tile([128, 4 * 1024], f16)
            x8_sb = main.tile([128, EB * T], f8)
            xr4_sb = main.tile([128, EB * T], f8)
            x8q_sb = main.tile([128, EB * T], f8)
            wA8_sb = main.tile([128, EB * 1024], f8)
            wAr4_sb = main.tile([128, EB * 1024], f8)
            wA8q_sb = main.tile([128, EB * 1024], f8)
            wB8_sb = main.tile([128, EB * 512], f8)
            wBr4_sb = main.tile([128, EB * 512], f8)
            wB8q_sb = main.tile([128, EB * 512], f8)

            nc.sync.dma_start(mask_sb[:], maskT[:])
            nc.sync.dma_start(id_sb[:], ident[:])
            nc.vector.memset(S_sb[:], 0.0)
            nc.vector.memset(S_f32[:], 0.0)
            ones_view = v_aug.rearrange("p (n h e) -> p n h e", h=HPC, e=65)[:, :, :, 64]
            nc.vector.memset(ones_view, 1.0)

            # Single strided DMA per tensor (HWDGE holds ~630ns serially per
            # DMA — count is the scarce resource). x-side tensors split in
            # T-halves so the tb0/tb1 stream lands before the full 2MB.
            def load_w(dst_sb, src, cols):
                nc.sync.dma_start(
                    dst_sb.rearrange("p (eb c) -> p eb c", eb=EB),
                    src.rearrange("(eb p) c -> p eb c", p=128))

            def load_x_half(dst_sb, src, h):
                sl = slice(h * (T // 2), (h + 1) * (T // 2))
                nc.sync.dma_start(
                    dst_sb.rearrange("p (eb t) -> p eb t", eb=EB)[:, :, sl],
                    src.rearrange("(eb p) t -> p eb t", p=128)[:, :, sl])

            load_w(wB8_sb, wB8, 512)
            load_x_half(x8_sb, x8, 0)
            load_w(wB8q_sb, wB8q, 512)
            load_x_half(xr4_sb, xr4, 0)
            load_x_half(x8q_sb, x8q, 0)
            load_w(wBr4_sb, wBr4, 512)
            load_w(wA8_sb, wA8, 1024)
            load_w(wA8q_sb, wA8q, 1024)
            load_w(wAr4_sb, wAr4, 1024)
            load_x_half(x8_sb, x8, 1)
            load_x_half(xr4_sb, xr4, 1)
            load_x_half(x8q_sb, x8q, 1)
            nc.sync.dma_start(wp_sb.rearrange("p (hp t) -> p hp t", hp=4),
                              wpT.rearrange("(hp p) t -> p hp t", p=128))

            # [p, eb, cols] views for DoubleRow k-tile pairs (dim1 = 2 slices)
            x8v = x8_sb.rearrange("p (eb t) -> p eb t", eb=EB)
            xr4v = xr4_sb.rearrange("p (eb t) -> p eb t", eb=EB)
            x8qv = x8q_sb.rearrange("p (eb t) -> p eb t", eb=EB)
            wA8v = wA8_sb.rearrange("p (eb c) -> p eb c", eb=EB)
            wAr4v = wAr4_sb.rearrange("p (eb c) -> p eb c", eb=EB)
            wA8qv = wA8q_sb.rearrange("p (eb c) -> p eb c", eb=EB)
            wB8v = wB8_sb.rearrange("p (eb c) -> p eb c", eb=EB)
            wBr4v = wBr4_sb.rearrange("p (eb c) -> p eb c", eb=EB)
            wB8qv = wB8q_sb.rearrange("p (eb c) -> p eb c", eb=EB)

            A_TERMS = [(wA8v, x8v), (wA8qv, xr4v), (wAr4v, x8qv)]
            B_TERMS = [(x8v, wB8v), (xr4v, wB8qv), (x8qv, wBr4v)]

            def proj_units(tb):
                units = []

                def a_unit(fb, tb=tb):
                    ps = psProj.tile([128, 512], f32, name="psa", tag="proj")
                    idx = 0
                    for wv, xv in A_TERMS:
                        for i in range(4):
                            nc.tensor.matmul(
                                ps[:],
                                wv[:, 2 * i:2 * i + 2, fb * 128:(fb + 1) * 128],
                                xv[:, 2 * i:2 * i + 2, tb * 512:(tb + 1) * 512],
                                start=(idx == 0), stop=(idx == 11), perf_mode=DR)
                            idx += 1
                    # phi(x) = relu(x) + min(exp(x), 1); PSUM holds 64*x
                    ex = phist.tile([128, 512], f16, name="ex", tag="ex")
                    nc.scalar.activation(ex[:], ps[:], AF.Exp, scale=1.0 / WS)
                    rl = phist.tile([128, 512], f16, name="rl", tag="rl")
                    nc.scalar.activation(rl[:], ps[:], AF.Relu, scale=1.0 / WS)
                    dst = qkT[:, fb * T + tb * 512: fb * T + tb * 512 + 512]
                    nc.vector.scalar_tensor_tensor(
                        dst, ex[:], 1.0, rl[:], ALU.min, ALU.add)

                def b_unit(n):
                    ps = psProj.tile([128, 512], f32, name="psb", tag="proj")
                    idx = 0
                    for xv, wv in B_TERMS:
                        for i in range(4):
                            nc.tensor.matmul(
                                ps[:],
                                xv[:, 2 * i:2 * i + 2, n * CH:(n + 1) * CH],
                                wv[:, 2 * i:2 * i + 2, :],
                                start=(idx == 0), stop=(idx == 11), perf_mode=DR)
                            idx += 1
                    src_ = ps.rearrange("p (h e) -> p h e", e=64)
                    dst = v_aug[:, n * HPC * 65:(n + 1) * HPC * 65] \
                        .rearrange("p (h e) -> p h e", e=65)[:, :, 0:64]
                    nc.scalar.mul(dst, src_, 1.0 / WS)

                def kt_unit(n):
                    kt = psTp.tile([128, 512], f16, name="ktp", tag="tp")
                    for fb4 in range(4):
                        nc.tensor.transpose(
                            kt[:, fb4 * 128:(fb4 + 1) * 128],
                            qkT[:, (4 + fb4) * T + n * CH:(4 + fb4) * T + n * CH + CH],
                            id_sb[:])
                    nc.vector.tensor_copy(
                        k_tok[:, n * 512:(n + 1) * 512], kt[:])

                if tb == 0:
                    for n in range(4):
                        units.append(lambda n=n: b_unit(n))
                    for fb in range(FB_QK):
                        units.append(lambda fb=fb: a_unit(fb))
                else:
                    for fb in range(FB_QK):
                        units.append(lambda fb=fb: a_unit(fb))
                    for n in range(tb * 4, tb * 4 + 4):
                        units.append(lambda n=n: b_unit(n))
                for n in range(tb * 4, tb * 4 + 4):
                    units.append(lambda n=n: kt_unit(n))
                return units

            def emit_p3(n):
                # output projection for token block n (needs yT_all of chunk n)
                for eo in range(2):
                    po = psProj.tile([128, 512], f32, name="pop", tag="proj")
                    for hp in range(4):
                        nc.tensor.matmul(
                            po[:],
                            yT_all[:, hp * T + n * CH: hp * T + n * CH + CH],
                            wp_sb[:, hp * 1024 + eo * 512: hp * 1024 + eo * 512 + 512],
                            start=(hp == 0), stop=(hp == 3))
                    ob = ost.tile([128, 512], f16, name="ob", tag="ob")
                    nc.scalar.copy(ob[:], po[:])
                    nc.sync.dma_start(
                        out[n * CH:(n + 1) * CH, eo * 512:(eo + 1) * 512], ob[:])

            def emit_chunk(n, filler=None):
                # scoresT, one [128,512] tile per parity half (bank-pure
                # row-groups; 2-buf rotation pipelines across chunks)
                scm = scmp.tile([128, HPC * CH], f16, name="scm", tag="scm")
                scgs = [psS.tile([128, 512], f32, name=f"scg{g}", tag="scg")
                        for g in range(2)]
                for hp in range(4):
                    for g in range(2):  # alternate row groups -> PE overlaps
                        b64 = g * 64
                        nc.tensor.matmul(
                            scgs[g][:, hp * CH:(hp + 1) * CH],
                            qkT[b64:b64 + 64,
                                (4 + hp) * T + n * CH:(4 + hp) * T + n * CH + CH],
                            qkT[b64:b64 + 64, hp * T + n * CH: hp * T + n * CH + CH],
                            start=True, stop=True)
                for g in range(2):
                    nc.vector.tensor_mul(scm[:, g * 512:(g + 1) * 512], scgs[g][:],
                                         mask_sb[:, g * 512:(g + 1) * 512])

                # PE filler while the mask/cast runs on DVE
                if n >= 1:
                    emit_p3(n - 1)
                if filler is not None:
                    filler()

                # num = scores @ v_aug + q @ S_aug (per parity half)
                nmb_g = []
                for g in range(2):
                    nmg = psN.tile([128, 512], f32, name="nmg", tag="nmg")
                    nmb_g.append(nmg)
                    for hp in range(4):
                        h = hp * 2 + g
                        b64 = g * 64
                        hc = hp * 128
                        qT_ap = qkT[b64:b64 + 64, hp * T + n * CH: hp * T + n * CH + CH]
                        va_ap = v_aug[:, (n * HPC + h) * 65:(n * HPC + h) * 65 + 65]
                        nc.tensor.matmul(nmg[:, hc: hc + 65],
                                         scm[:, g * 512 + hc: g * 512 + hc + CH], va_ap,
                                         start=True, stop=False)
                        nc.tensor.matmul(nmg[:, hc: hc + 65], qT_ap,
                                         S_sb[b64:b64 + 64, hp * 128: hp * 128 + 65],
                                         start=False, stop=True)

                # state deltas + f32 accumulate + f16 snapshot (snap on Pool)
                sdb = psSd.tile([128, 512], f32, name="sdb", tag="sdb")
                for h in range(HPC):
                    b64 = (h % 2) * 64
                    hp = h // 2
                    va_ap = v_aug[:, (n * HPC + h) * 65:(n * HPC + h) * 65 + 65]
                    nc.tensor.matmul(
                        sdb[b64:b64 + 64, hp * 128: hp * 128 + 65],
                        k_tok[:, n * 512 + h * 64: n * 512 + h * 64 + 64],
                        va_ap, start=True, stop=True)
                Sf_v = S_f32.rearrange("p (g e) -> p g e", e=CH)[:, :, 0:65]
                Sb_v = S_sb.rearrange("p (g e) -> p g e", e=CH)[:, :, 0:65]
                sd_v = sdb.rearrange("p (g e) -> p g e", e=CH)[:, :, 0:65]
                nc.vector.tensor_add(Sf_v, Sf_v, sd_v)
                nc.vector.tensor_copy(Sb_v, Sf_v)

                # y = num / den, per parity half
                yb = st2.tile([128, 512], f16, name="yb", tag="yb")
                yb_v = yb.rearrange("p (hh two e) -> p hh two e", two=2, e=64)
                for g in range(2):
                    half = nmb_g[g].rearrange("p (hh e) -> p hh e", e=CH)
                    rcp = st2.tile([128, 4], f32, name=f"rcp{g}", tag=f"rcp{g}")
                    nc.vector.reciprocal(rcp[:], half[:, :, 64])
                    nc.vector.tensor_mul(
                        yb_v[:, :, g, :],
                        half[:, :, 0:64],
                        rcp[:, :, None].broadcast_to([128, 4, 64]))

                # yT via PE transpose (head pairs) -> yT_all
                ytp = psTp.tile([128, 512], f16, name="ytp", tag="tp")
                for hp in range(4):
                    nc.tensor.transpose(ytp[:, hp * 128:(hp + 1) * 128],
                                        yb[:, hp * CH:(hp + 1) * CH], id_sb[:])
                nc.vector.tensor_copy(
                    yT_all.rearrange("p (hp t) -> p hp t", hp=4)[:, :, n * CH:(n + 1) * CH],
                    ytp.rearrange("p (hp e) -> p hp e", hp=4))

            for u in proj_units(0):
                u()
            units = []
            for tb in range(1, TB):
                units.extend(proj_units(tb))
            state = {"ui": 0}

            def pace(target):
                while state["ui"] < min(target, len(units)):
                    units[state["ui"]]()
                    state["ui"] += 1

            for n in range(NCH):
                pace(16 * (n // 4))          # hard dep: chunk n needs its tb
                emit_chunk(n, lambda: pace(7 * (n + 1)))
            emit_p3(NCH - 1)

    nc.compile()
    return nc


def _split3(a, hi_dt=ml_dtypes.float8_e4m3fn):
    """fp8 hi / 4x-residual / quarter triplet of a float32 array."""
    a8 = a.astype(hi_dt)
    ar4 = ((a - a8.astype(np.float32)) * 4.0).astype(hi_dt)
    a8q = (a * 0.25).astype(hi_dt)
    return a8, ar4, a8q


def _prep_core_inputs(x, w_attn, w_proj, core):
    b, hg = core // 2, core % 2
    s = slice(hg * 512, (hg + 1) * 512)
    xT = np.ascontiguousarray(x[b].T)
    wA = np.ascontiguousarray(
        np.concatenate([w_attn[s, :], w_attn[E + hg * 512: E + (hg + 1) * 512, :]],
                       0).T) * WS
    wB = np.ascontiguousarray(
        w_attn[2 * E + hg * 512: 2 * E + (hg + 1) * 512, :].T) * WS
    x8, xr4, x8q = _split3(xT)
    wA8, wAr4, wA8q = _split3(wA)
    wB8, wBr4, wB8q = _split3(wB)
    wpT_ = np.ascontiguousarray(w_proj[:, s].T).astype(np.float16)
    mask1 = np.triu(np.ones((CH, CH), dtype=np.float32))
    maskT = np.tile(mask1, (1, HPC)).astype(np.float16)
    ident = np.eye(CH, dtype=np.float32).astype(np.float16)
    return {"x8": x8, "xr4": xr4, "x8q": x8q,
            "wA8": wA8, "wAr4": wAr4, "wA8q": wA8q,
            "wB8": wB8, "wBr4": wBr4, "wB8q": wB8q,
            "wpT": wpT_, "maskT": maskT, "ident": ident}


def kernel(x, w_attn, w_proj, _trace=False):
    from concourse.bass_utils import run_bass_kernel_spmd

    if "nc" not in _cache:
        _cache["nc"] = _build()
    nc = _cache["nc"]

    x = np.asarray(x, dtype=np.float32)
    w_attn = np.asarray(w_attn, dtype=np.float32)
    w_proj = np.asarray(w_proj, dtype=np.float32)

    in_maps = [_prep_core_inputs(x, w_attn, w_proj, c) for c in range(N_CORES)]
    res = run_bass_kernel_spmd(nc, in_maps, core_ids=list(range(N_CORES)),
                               trace=_trace)
    _cache["last_results"] = res

    out = np.empty((B, T, E), dtype=np.float32)
    for b in range(B):
        out[b] = (res.results[2 * b]["out"].astype(np.float32)
                  + res.results[2 * b + 1]["out"].astype(np.float32))
    return out
